# revision 15
# baseline (speedup 1.0000x reference)
"""Trainium2 Bass kernel for nn_LsunIntermediateRotation2dLayer.

Computation: X [64, 256, 256, 16] fp32; per spatial block (r, c) an 8x8
orthonormal matrix R (28 cascaded Givens rotations + mu row signs) is applied
as R^T to channels 8:16; channels 0:8 pass through.

Sharding: data-parallel over rows r — 8 cores x 32 rows each (angles/mus
shard with blocks). Each core runs an identical Bass program on its slice.

Design notes (correctness gate is 2e-2 so bf16 matmul is fine, err ~4e-3):
  - no range wrap: |angles| <= ~0.6 << pi, cos(x) = sin(x + pi/2) < pi
  - Givens cascade as 3 DVE ops per rotation using a row-pair view, a
    negative-stride swapped view, and a sign-packed sin table:
        P = [rt; rb] * c,  Q = [rb; rt] * [-s; +s],  [rt'; rb'] = P + Q
    split across vector (44 u-groups) and gpsimd (20) in parallel
  - relayout between the two xbar transposes loops over uq so writes are
    contiguous (bf16 strided sub-word writes are pathologically slow)
  - main loop engine assignment decouples pipeline stages so iteration i's
    drain never blocks iteration i+1's cast through one engine's in-order
    stream: vector = cast + both bd mask-scatters, gpsimd = PSUM drain,
    scalar = store triggers only, sync = load + YS-transpose triggers
  - io pool is deep (10 bufs) so X prefetch hides the R-build prefix
"""
import sys

if '/opt/trn_rl_repo' not in sys.path:
    sys.path.insert(0, '/opt/trn_rl_repo')

import math

import numpy as np

N_CORES = 8
NSAMP, NROWS, NCOLS, NCH = 64, 256, 256, 16
RR = NROWS // N_CORES          # 32 rows per core
NBLK = RR * NCOLS              # 8192 blocks per core
NU = NBLK // 128               # 64 partition-tile groups
PS = 8
NANG = 28

_CACHE = {}


def _build_nc(rr_count=RR):
    import concourse.bass as bass
    import concourse.tile as tile
    from concourse import bacc, mybir

    nblk = rr_count * NCOLS
    nu = nblk // 128
    nuq = nu // 16

    f32 = mybir.dt.float32
    bf16 = mybir.dt.bfloat16
    mult = mybir.AluOpType.mult
    add = mybir.AluOpType.add
    Sin = mybir.ActivationFunctionType.Sin

    U_V = max(1, (nu * 11) // 16)  # u groups on vector; rest on gpsimd
    U_G = nu - U_V

    nc = bacc.Bacc("TRN2", target_bir_lowering=False)
    X_d = nc.declare_dram_parameter("X", [NSAMP, rr_count, NCOLS, NCH], f32, isOutput=False)
    ang_d = nc.declare_dram_parameter("angles", [nblk, NANG], f32, isOutput=False)
    mus_d = nc.declare_dram_parameter("mus", [nblk, PS], f32, isOutput=False)
    out_d = nc.declare_dram_parameter("out", [NSAMP, rr_count, NCOLS, NCH], f32, isOutput=True)
    mask_d = nc.declare_dram_parameter("mask", [128, 128], bf16, isOutput=False)

    with tile.TileContext(nc) as tc:
        with (
            tc.tile_pool(name="rkeep", bufs=1) as rk,
            tc.tile_pool(name="io", bufs=8) as iop,
            tc.tile_pool(name="stage", bufs=4) as stp,
            tc.tile_pool(name="bdp", bufs=3) as bdp,
            tc.tile_pool(name="psum", bufs=4, space="PSUM") as psp,
        ):
            from contextlib import ExitStack
            rp_stack = ExitStack()
            rp = rp_stack.enter_context(tc.tile_pool(name="rbuild", bufs=1))
            # ---------------- R build phase ----------------
            A = rp.tile([128, nu, NANG], f32, tag="A")
            MU = rp.tile([128, nu, PS], f32, tag="MU")
            nc.sync.dma_start(A[:], ang_d[:].rearrange("(u p) k -> p u k", p=128))
            nc.sync.dma_start(MU[:], mus_d[:].rearrange("(u p) k -> p u k", p=128))

            S = rp.tile([128, nu, NANG], f32, tag="S")
            C = rp.tile([128, nu, NANG], f32, tag="C")
            nc.scalar.activation(S[:], A[:], Sin)
            nc.vector.tensor_scalar(out=A[:], in0=A[:], scalar1=math.pi / 2,
                                    scalar2=None, op0=add)
            nc.scalar.activation(C[:], A[:], Sin)
            # sign-packed sin: SP[:, :, 0, k] = -s_k, SP[:, :, 1, k] = +s_k
            SP = rp.tile([128, nu, 2, NANG], f32, tag="SP")
            for eng, u0, un in ((nc.vector, 0, U_V), (nc.gpsimd, U_V, U_G)):
                eng.tensor_scalar(out=SP[:, u0:u0 + un, 0], in0=S[:, u0:u0 + un],
                                  scalar1=-1.0, scalar2=None, op0=mult)
                eng.tensor_scalar(out=SP[:, u0:u0 + un, 1], in0=S[:, u0:u0 + un],
                                  scalar1=1.0, scalar2=None, op0=mult)

            R = rp.tile([128, nu, PS, PS], f32, tag="R")
            for eng, u0, un in ((nc.vector, 0, U_V), (nc.gpsimd, U_V, U_G)):
                eng.memset(R[:, u0:u0 + un], 0.0)
                for j in range(PS):
                    eng.memset(R[:, u0:u0 + un, j, j], 1.0)

            # Givens cascade: 3 ops per rotation, engine-split by u range.
            Rb = rp.tile([128, PS, nu, PS], bf16, tag="Rb")  # (j, U, i) layout
            vb = Rb[:].transpose([0, 2, 1, 3])               # [128, nu, j, i] view
            for eng, u0, un in ((nc.vector, 0, U_V), (nc.gpsimd, U_V, U_G)):
                P = rp.tile([128, un, 2, PS], f32, tag=f"P_{u0}")
                Q = rp.tile([128, un, 2, PS], f32, tag=f"Q_{u0}")
                Ru = R[:, u0:u0 + un]
                Cs = C[:, u0:u0 + un]
                SPs = SP[:, u0:u0 + un]
                k = 0
                for t in range(PS - 1):
                    for b in range(t + 1, PS):
                        st = b - t
                        pair = Ru[:, :, t:b + 1:st, :]            # rows (t, b)
                        Cb = Cs[:, :, k:k + 1].unsqueeze(2).broadcast_to(
                            (128, un, 2, PS))
                        SPn = SPs[:, :, 0, k:k + 1].broadcast_to((128, un, PS))
                        SPp = SPs[:, :, 1, k:k + 1].broadcast_to((128, un, PS))
                        eng.tensor_tensor(out=P[:], in0=pair, in1=Cb, op=mult)
                        # Q rows in (t, b) output order: [-s*rb ; +s*rt]
                        eng.tensor_tensor(out=Q[:, :, 0], in0=Ru[:, :, b, :],
                                          in1=SPn, op=mult)
                        eng.tensor_tensor(out=Q[:, :, 1], in0=Ru[:, :, t, :],
                                          in1=SPp, op=mult)
                        eng.tensor_tensor(out=pair, in0=P[:], in1=Q[:], op=add)
                        k += 1
                # row signs fused with the bf16 downcast into (j, U, i) layout
                eng.tensor_tensor(
                    out=vb[:, u0:u0 + un], in0=Ru,
                    in1=MU[:, u0:u0 + un].unsqueeze(3).broadcast_to(
                        (128, un, PS, PS)), op=mult)

            # double transpose: [blk, (j,U,i)] -> [(U16,i), (j,Uq,blk)]
            #   -> relayout -> [(U16,i), (Uq,o,g,j)] -> [(g,j), ((Uq,o),(U16,i))]
            o1 = rp.tile([128, PS * nuq, 128], bf16, tag="o1")
            nc.sync.dma_start(o1[:], Rb[:], transpose=True)
            tmp = rp.tile([128, nuq, 8, 16, PS], bf16, tag="tmp")
            o1v = o1[:].rearrange("p (j uq) (o g) -> p j uq o g", j=PS, o=8)
            for uq in range(nuq):
                # contiguous write [o, g, j]; strided read from (j, o, g)
                nc.vector.tensor_copy(tmp[:, uq],
                                      o1v[:, :, uq].transpose([0, 2, 3, 1]))
            R2 = rk.tile([128, nuq * 8, 128], bf16, tag="R2")
            nc.sync.dma_start(R2[:], tmp[:], transpose=True)

            MASKt = rk.tile([128, 128], bf16, tag="MASK")
            nc.sync.dma_start(MASKt[:], mask_d[:])
            maskb = MASKt[:].rearrange("p (g i) -> p g i", g=16)

            rp_stack.close()
            # ---------------- main loop ----------------
            r2v = R2[:].rearrange(
                "p (uq o) (u16 i) -> p uq o u16 i", uq=nuq, u16=16)

            for rr in range(rr_count):
                T0 = iop.tile([128, 128, NCH], f32, tag="T0")
                for h in range(2):
                    nc.sync.dma_start(
                        T0[h * 64:(h + 1) * 64, :, :],
                        X_d[:, rr, h * 128:(h + 1) * 128, :])

                # rotation channels -> bf16 (vector)
                Ab = stp.tile([128, 128, PS], bf16, tag="Ab")
                nc.vector.tensor_copy(Ab[:], T0[:, :, 8:16])

                YS = stp.tile([128, 8, 128], bf16, tag="YS")
                nc.scalar.dma_start(YS[:], Ab[:], transpose=True)

                # block-diag weights: bd[8g+j, o, h, 8g'+i] =
                #   mask[g==g'] * R2[8g+j, (Uq, o), (U16(h), i)]
                bd = bdp.tile([128, 8, 2, 128], bf16, tag="bd")
                uq, u16 = (2 * rr) // 16, (2 * rr) % 16
                # bd depends only on prefix data (R2/mask) — keep it on
                # gpsimd, off the load->cast chain, so PE never transitively
                # waits on far-future loads through an engine stream.
                for h in range(2):
                    in0 = (r2v[:, uq, :, u16 + h, :]
                           .unsqueeze(2)
                           .broadcast_to((128, 8, 16, PS)))
                    in1 = maskb.unsqueeze(1).broadcast_to((128, 8, 16, PS))
                    nc.gpsimd.tensor_tensor(
                        out=bd[:, :, h, :].rearrange(
                            "p o (g i) -> p o g i", g=16),
                        in0=in0, in1=in1, op=mult)

                ps = psp.tile([128, 8, 128], f32, tag="ps")
                for o in range(8):
                    for h in range(2):
                        m_sl = slice(h * 64, h * 64 + 64)
                        nc.tensor.matmul(ps[m_sl, o, :], YS[:, o, m_sl],
                                         bd[:, o, h, :], start=True, stop=True)

                # drain PSUM into T0's rotation-channel slots (scalar; its
                # stream is drain_i, store_i, drain_i+1 ... — no coupling)
                t0v = T0[:].rearrange("p (o g) ch -> p o g ch", g=16)
                psv = ps[:].rearrange("p o (g i) -> p o g i", g=16)
                nc.scalar.activation(t0v[:, :, :, 8:16], psv[:],
                                     mybir.ActivationFunctionType.Copy)

                for h in range(2):
                    nc.scalar.dma_start(
                        out_d[:, rr, h * 128:(h + 1) * 128, :],
                        T0[h * 64:(h + 1) * 64, :, :])

    nc.finalize()
    return nc


def _get_nc():
    if "nc" not in _CACHE:
        _CACHE["nc"] = _build_nc()
    return _CACHE["nc"]


def block_diag_mask():
    import ml_dtypes
    m = np.kron(np.eye(16, dtype=np.float32), np.ones((8, 8), dtype=np.float32))
    return np.ascontiguousarray(m.astype(ml_dtypes.bfloat16))


def kernel(X, angles, mus):
    from concourse.bass_utils import run_bass_kernel_spmd

    X = np.ascontiguousarray(X, dtype=np.float32)
    angles = np.ascontiguousarray(angles, dtype=np.float32)
    mus = np.ascontiguousarray(mus, dtype=np.float32)

    nc = _get_nc()
    mask = block_diag_mask()
    in_maps = []
    for c in range(N_CORES):
        in_maps.append({
            "X": np.ascontiguousarray(X[:, c * RR:(c + 1) * RR]),
            "angles": np.ascontiguousarray(angles[c * NBLK:(c + 1) * NBLK]),
            "mus": np.ascontiguousarray(mus[c * NBLK:(c + 1) * NBLK]),
            "mask": mask,
        })
    res = run_bass_kernel_spmd(nc, in_maps, list(range(N_CORES)))
    out = np.concatenate([res.results[c]["out"] for c in range(N_CORES)], axis=1)
    return out


# revision 23
# speedup vs baseline: 1.3425x; 1.3425x over previous
"""Trainium2 Bass kernel for nn_LsunIntermediateRotation2dLayer.

Computation: X [64, 256, 256, 16] fp32; per spatial block (r, c) an 8x8
orthonormal matrix R (28 cascaded Givens rotations + mu row signs) is applied
as R^T to channels 8:16; channels 0:8 pass through.

Sharding: data-parallel over rows r — 8 cores x 32 rows each (angles/mus
shard with blocks). Each core runs an identical Bass program on its slice.

Design notes (correctness gate is 2e-2 so bf16 matmul is fine, err ~4e-3):
  - no range wrap: |angles| <= ~0.6 << pi, cos(x) = sin(x + pi/2) < pi
  - Givens cascade as 3 DVE ops per rotation using a row-pair view, a
    negative-stride swapped view, and a sign-packed sin table:
        P = [rt; rb] * c,  Q = [rb; rt] * [-s; +s],  [rt'; rb'] = P + Q
    split across vector (44 u-groups) and gpsimd (20) in parallel
  - relayout between the two xbar transposes loops over uq so writes are
    contiguous (bf16 strided sub-word writes are pathologically slow)
  - main loop engine assignment decouples pipeline stages so iteration i's
    drain never blocks iteration i+1's cast through one engine's in-order
    stream: vector = cast + both bd mask-scatters, gpsimd = PSUM drain,
    scalar = store triggers only, sync = load + YS-transpose triggers
  - io pool is deep (10 bufs) so X prefetch hides the R-build prefix
"""
import sys

if '/opt/trn_rl_repo' not in sys.path:
    sys.path.insert(0, '/opt/trn_rl_repo')

import math

import numpy as np

N_CORES = 8
NSAMP, NROWS, NCOLS, NCH = 64, 256, 256, 16
RR = NROWS // N_CORES          # 32 rows per core
NBLK = RR * NCOLS              # 8192 blocks per core
NU = NBLK // 128               # 64 partition-tile groups
PS = 8
NANG = 28

_CACHE = {}


def _build_nc(rr_count=RR):
    import concourse.bass as bass
    import concourse.tile as tile
    from concourse import bacc, mybir

    nblk = rr_count * NCOLS
    nu = nblk // 128
    nuq = nu // 16

    f32 = mybir.dt.float32
    bf16 = mybir.dt.bfloat16
    mult = mybir.AluOpType.mult
    add = mybir.AluOpType.add
    Sin = mybir.ActivationFunctionType.Sin

    U_V = max(1, (nu * 11) // 16)  # u groups on vector; rest on gpsimd
    U_G = nu - U_V

    nc = bacc.Bacc("TRN2", target_bir_lowering=False)
    X_d = nc.declare_dram_parameter("X", [NSAMP, rr_count, NCOLS, NCH], f32, isOutput=False)
    ang_d = nc.declare_dram_parameter("angles", [nblk, NANG], f32, isOutput=False)
    mus_d = nc.declare_dram_parameter("mus", [nblk, PS], f32, isOutput=False)
    out_d = nc.declare_dram_parameter("out", [NSAMP, rr_count, NCOLS, NCH], f32, isOutput=True)
    mask_d = nc.declare_dram_parameter("mask", [128, 128], bf16, isOutput=False)
    ident_d = nc.declare_dram_parameter("ident", [128, 128], bf16, isOutput=False)

    with tile.TileContext(nc) as tc:
        with (
            tc.tile_pool(name="rkeep", bufs=1) as rk,
            tc.tile_pool(name="io", bufs=8) as iop,
            tc.tile_pool(name="stage", bufs=4) as stp,
            tc.tile_pool(name="bdp", bufs=3) as bdp,
            tc.tile_pool(name="psum", bufs=3, space="PSUM") as psp,
            tc.tile_pool(name="psumt", bufs=2, space="PSUM") as pst,
        ):
            from contextlib import ExitStack
            rp_stack = ExitStack()
            rp = rp_stack.enter_context(tc.tile_pool(name="rbuild", bufs=1))
            # ---------------- R build phase ----------------
            A = rp.tile([128, nu, NANG], f32, tag="A")
            MU = rp.tile([128, nu, PS], f32, tag="MU")
            nc.sync.dma_start(A[:], ang_d[:].rearrange("(u p) k -> p u k", p=128))
            nc.sync.dma_start(MU[:], mus_d[:].rearrange("(u p) k -> p u k", p=128))

            S = rp.tile([128, nu, NANG], f32, tag="S")
            C = rp.tile([128, nu, NANG], f32, tag="C")
            nc.scalar.activation(S[:], A[:], Sin)
            nc.vector.tensor_scalar(out=A[:], in0=A[:], scalar1=math.pi / 2,
                                    scalar2=None, op0=add)
            nc.scalar.activation(C[:], A[:], Sin)
            # sign-packed sin: SP[:, :, 0, k] = -s_k, SP[:, :, 1, k] = +s_k
            SP = rp.tile([128, nu, 2, NANG], f32, tag="SP")
            for eng, u0, un in ((nc.vector, 0, U_V), (nc.gpsimd, U_V, U_G)):
                eng.tensor_scalar(out=SP[:, u0:u0 + un, 0], in0=S[:, u0:u0 + un],
                                  scalar1=-1.0, scalar2=None, op0=mult)
                eng.tensor_scalar(out=SP[:, u0:u0 + un, 1], in0=S[:, u0:u0 + un],
                                  scalar1=1.0, scalar2=None, op0=mult)

            R = rp.tile([128, nu, PS, PS], f32, tag="R")
            for eng, u0, un in ((nc.vector, 0, U_V), (nc.gpsimd, U_V, U_G)):
                eng.memset(R[:, u0:u0 + un], 0.0)
                for j in range(PS):
                    eng.memset(R[:, u0:u0 + un, j, j], 1.0)

            # Givens cascade: 3 ops per rotation, engine-split by u range.
            Rb = rp.tile([128, PS, nu, PS], bf16, tag="Rb")  # (j, U, i) layout
            vb = Rb[:].transpose([0, 2, 1, 3])               # [128, nu, j, i] view
            for eng, u0, un in ((nc.vector, 0, U_V), (nc.gpsimd, U_V, U_G)):
                P = rp.tile([128, un, 2, PS], f32, tag=f"P_{u0}")
                Q = rp.tile([128, un, 2, PS], f32, tag=f"Q_{u0}")
                Ru = R[:, u0:u0 + un]
                Cs = C[:, u0:u0 + un]
                SPs = SP[:, u0:u0 + un]
                k = 0
                for t in range(PS - 1):
                    for b in range(t + 1, PS):
                        st = b - t
                        pair = Ru[:, :, t:b + 1:st, :]            # rows (t, b)
                        Cb = Cs[:, :, k:k + 1].unsqueeze(2).broadcast_to(
                            (128, un, 2, PS))
                        SPn = SPs[:, :, 0, k:k + 1].broadcast_to((128, un, PS))
                        SPp = SPs[:, :, 1, k:k + 1].broadcast_to((128, un, PS))
                        eng.tensor_tensor(out=P[:], in0=pair, in1=Cb, op=mult)
                        # Q rows in (t, b) output order: [-s*rb ; +s*rt]
                        eng.tensor_tensor(out=Q[:, :, 0], in0=Ru[:, :, b, :],
                                          in1=SPn, op=mult)
                        eng.tensor_tensor(out=Q[:, :, 1], in0=Ru[:, :, t, :],
                                          in1=SPp, op=mult)
                        eng.tensor_tensor(out=pair, in0=P[:], in1=Q[:], op=add)
                        k += 1
                # row signs fused with the bf16 downcast into (j, U, i) layout
                eng.tensor_tensor(
                    out=vb[:, u0:u0 + un], in0=Ru,
                    in1=MU[:, u0:u0 + un].unsqueeze(3).broadcast_to(
                        (128, un, PS, PS)), op=mult)

            # double transpose: [blk, (j,U,i)] -> [(U16,i), (j,Uq,blk)]
            #   -> relayout -> [(U16,i), (Uq,o,g,j)] -> [(g,j), ((Uq,o),(U16,i))]
            o1 = rp.tile([128, PS * nuq, 128], bf16, tag="o1")
            nc.sync.dma_start(o1[:], Rb[:], transpose=True)
            tmp = rp.tile([128, nuq, 8, 16, PS], bf16, tag="tmp")
            o1v = o1[:].rearrange("p (j uq) (o g) -> p j uq o g", j=PS, o=8)
            for uq in range(nuq):
                # contiguous write [o, g, j]; strided read from (j, o, g)
                nc.vector.tensor_copy(tmp[:, uq],
                                      o1v[:, :, uq].transpose([0, 2, 3, 1]))
            R2 = rk.tile([128, nuq * 8, 128], bf16, tag="R2")
            nc.sync.dma_start(R2[:], tmp[:], transpose=True)

            MASKt = rk.tile([128, 128], bf16, tag="MASK")
            nc.sync.dma_start(MASKt[:], mask_d[:])
            maskb = MASKt[:].rearrange("p (g i) -> p g i", g=16)
            IDT = rk.tile([128, 128], bf16, tag="IDT")
            nc.sync.dma_start(IDT[:], ident_d[:])

            rp_stack.close()
            # ---------------- main loop ----------------
            r2v = R2[:].rearrange(
                "p (uq o) (u16 i) -> p uq o u16 i", uq=nuq, u16=16)

            for rr in range(rr_count):
                T0 = iop.tile([128, 128, NCH], f32, tag="T0")
                for h in range(2):
                    nc.sync.dma_start(
                        T0[h * 64:(h + 1) * 64, :, :],
                        X_d[:, rr, h * 128:(h + 1) * 128, :])

                # rotation channels -> bf16 (vector)
                Ab = stp.tile([128, 128, PS], bf16, tag="Ab")
                nc.vector.tensor_copy(Ab[:], T0[:, :, 8:16])

                # transpose [(h,n), (c,j)] -> [(g,j), o, (h,n)] on the PE
                # (8 128x128 chunks), then PSUM -> SBUF on vector.  Keeps the
                # per-iter DMA count at 2 so queue-completion semaphore lanes
                # don't serialize unrelated transfers.
                abf = Ab[:].rearrange("p c j -> p (c j)")
                ysp = pst.tile([128, 8, 128], bf16, tag="ysp")
                for q in range(8):
                    nc.tensor.transpose(ysp[:, q, :],
                                        abf[:, q * 128:(q + 1) * 128], IDT[:])
                YS = stp.tile([128, 8, 128], bf16, tag="YS")
                nc.vector.tensor_copy(YS[:], ysp[:])

                # block-diag weights: bd[8g+j, o, h, 8g'+i] =
                #   mask[g==g'] * R2[8g+j, (Uq, o), (U16(h), i)]
                bd = bdp.tile([128, 8, 2, 128], bf16, tag="bd")
                uq, u16 = (2 * rr) // 16, (2 * rr) % 16
                # bd depends only on prefix data (R2/mask) — keep it on
                # gpsimd, off the load->cast chain, so PE never transitively
                # waits on far-future loads through an engine stream.
                for h in range(2):
                    in0 = (r2v[:, uq, :, u16 + h, :]
                           .unsqueeze(2)
                           .broadcast_to((128, 8, 16, PS)))
                    in1 = maskb.unsqueeze(1).broadcast_to((128, 8, 16, PS))
                    nc.gpsimd.tensor_tensor(
                        out=bd[:, :, h, :].rearrange(
                            "p o (g i) -> p o g i", g=16),
                        in0=in0, in1=in1, op=mult)

                ps = psp.tile([128, 8, 128], f32, tag="ps")
                for o in range(8):
                    for h in range(2):
                        m_sl = slice(h * 64, h * 64 + 64)
                        nc.tensor.matmul(ps[m_sl, o, :], YS[:, o, m_sl],
                                         bd[:, o, h, :], start=True, stop=True)

                # drain PSUM into T0's rotation-channel slots (scalar; its
                # stream is drain_i, store_i, drain_i+1 ... — no coupling)
                t0v = T0[:].rearrange("p (o g) ch -> p o g ch", g=16)
                psv = ps[:].rearrange("p o (g i) -> p o g i", g=16)
                nc.scalar.activation(t0v[:, :, :, 8:16], psv[:],
                                     mybir.ActivationFunctionType.Copy)

                for h in range(2):
                    nc.scalar.dma_start(
                        out_d[:, rr, h * 128:(h + 1) * 128, :],
                        T0[h * 64:(h + 1) * 64, :, :])

    nc.finalize()
    return nc


def _get_nc():
    if "nc" not in _CACHE:
        _CACHE["nc"] = _build_nc()
    return _CACHE["nc"]


def block_diag_mask():
    import ml_dtypes
    m = np.kron(np.eye(16, dtype=np.float32), np.ones((8, 8), dtype=np.float32))
    return np.ascontiguousarray(m.astype(ml_dtypes.bfloat16))


def identity128():
    import ml_dtypes
    return np.ascontiguousarray(np.eye(128, dtype=np.float32).astype(ml_dtypes.bfloat16))


def make_in_maps(X, angles, mus):
    mask = block_diag_mask()
    ident = identity128()
    in_maps = []
    for c in range(N_CORES):
        in_maps.append({
            "X": np.ascontiguousarray(X[:, c * RR:(c + 1) * RR]),
            "angles": np.ascontiguousarray(angles[c * NBLK:(c + 1) * NBLK]),
            "mus": np.ascontiguousarray(mus[c * NBLK:(c + 1) * NBLK]),
            "mask": mask,
            "ident": ident,
        })
    return in_maps


def kernel(X, angles, mus):
    from concourse.bass_utils import run_bass_kernel_spmd

    X = np.ascontiguousarray(X, dtype=np.float32)
    angles = np.ascontiguousarray(angles, dtype=np.float32)
    mus = np.ascontiguousarray(mus, dtype=np.float32)

    nc = _get_nc()
    in_maps = make_in_maps(X, angles, mus)
    res = run_bass_kernel_spmd(nc, in_maps, list(range(N_CORES)))
    out = np.concatenate([res.results[c]["out"] for c in range(N_CORES)], axis=1)
    return out
